# revision 35
# baseline (speedup 1.0000x reference)
"""MoE block (B=16,N=1024,C=768,E=8,H=192,D=4,K=2) on 8 NeuronCores.

Strategy: data-parallel over B (2 samples/core). Everything is laid out to
minimize DMA bytes/instructions (the cost-model bottleneck) and PE column
traffic:

  - xT fp16 (pre-transposed on host) serves gating (needs ~11 mantissa bits
    for exact top-2), the fc2-side residual add, and is the only dense fp16
    copy of x. A second fp8(e4m3) copy feeds fc1 in DoubleRow mode.
  - Gating matmuls use tiny output columns (out [128 tokens, 16]) so PE cost
    is ~16 cols/chunk instead of 512. Token reduction of clean/noise logits
    is a ones-vector matmul accumulated in PSUM.
  - Top-2 gate VALUES are constants (softmax of (d)/(d+1e-6) over 2 entries
    saturates: g1=sigmoid(1), g2=1-g1, exact to <1e-6 for any non-degenerate
    gap), so they are folded into two pre-scaled fc2 copies in the packed
    weight table; the top-1/top-2 gathers select the right copy. This removes
    all per-element gate multiplies.
  - fc1/fc2 run in fp8e4 DoubleRow (0.5 cyc/col, 256-deep contraction);
    gelu is applied PSUM->fp8 hT directly on the scalar engine.
  - Output is computed in transposed [C, N] layout so the residual is one
    more matmul (identity x xT16) accumulated into the fc2 PSUM; host
    transposes back. Output dtype fp16.
"""
import numpy as np
import ml_dtypes

import concourse.bass as bass
import concourse.mybir as mybir
import concourse.tile as tile
from concourse import bacc
from concourse.bass_utils import run_bass_kernel_spmd

f16 = np.float16
f32 = np.float32
e4 = ml_dtypes.float8_e4m3
AF = mybir.ActivationFunctionType
ALU = mybir.AluOpType
PM = mybir.MatmulPerfMode
dt = mybir.dt

B, N, C = 16, 1024, 768
E, H, D, TOPK = 8, 192, 4, 2
NCORES = 8
SPC = B // NCORES          # samples per core = 2
CK = C // 128              # 6 channel chunks
TCH = N // 128             # 8 token chunks
NT = N // 512              # 2 n-chunks for the 512-wide MLP matmuls
PCK = 24 * 128             # packed weight row: 12 fc1 blocks + 12 fc2 blocks
G1 = float(1.0 / (1.0 + np.exp(-1.0)))
G2 = 1.0 - G1
# softplus(r) = r/2 + g(r/2), g(y)=ln(2cosh y) ~= C0 + C1*y*tanh(C2*y) + C3*y^2
# (fit on |y|<=1.8, max err 2.3e-4; raw logits here stay within |y|<=1.25).
# Keeps the scalar engine on the single gelu table (tanh lives there too).
SP_C0, SP_C1, SP_C2, SP_C3 = (0.6932338862378958, 0.5501889808219406,
                              0.7575131375050952, 0.08185888665593381)

_cache = {}


def _build(reps=1, dbg=False):
    key = ("nc", reps, dbg)
    if key in _cache:
        return _cache[key]
    nc = bacc.Bacc("TRN2", target_bir_lowering=False, debug=False,
                   num_devices=NCORES)

    xt16_d = nc.dram_tensor("xt16", [SPC, CK, 128, N], dt.float16, kind="ExternalInput").ap()
    xt8_d = nc.dram_tensor("xt8", [SPC, CK, 128, N], dt.float8e4, kind="ExternalInput").ap()
    gw_d = nc.dram_tensor("gw16", [SPC, 128, CK * 16], dt.float16, kind="ExternalInput").ap()
    ep_d = nc.dram_tensor("eps16", [SPC, 128, TCH * 8], dt.float16, kind="ExternalInput").ap()
    wp_d = nc.dram_tensor("wpack", [2 * E * 128, PCK], dt.float8e4, kind="ExternalInput").ap()
    y_d = nc.dram_tensor("yT", [SPC, CK, 128, N], dt.float16, kind="ExternalOutput").ap()
    if dbg:
        dcomb_d = nc.dram_tensor("dcomb", [SPC, 128, TCH * 16], dt.float32, kind="ExternalOutput").ap()
        dews_d = nc.dram_tensor("dews", [SPC, 128, 8], dt.float32, kind="ExternalOutput").ap()
        dmi_d = nc.dram_tensor("dmi", [SPC, 128, 8], dt.uint32, kind="ExternalOutput").ap()
        dwt_d = nc.dram_tensor("dwt", [SPC, TOPK, 128, PCK], dt.float8e4, kind="ExternalOutput").ap()
        dht_d = nc.dram_tensor("dht", [SPC, TOPK, 128, 2 * N], dt.float8e4, kind="ExternalOutput").ap()

    with tile.TileContext(nc) as tc:
        with tc.tile_pool(name="const", bufs=1) as cp, \
             tc.tile_pool(name="xt", bufs=2) as xtp, \
             tc.tile_pool(name="gate", bufs=2) as gp, \
             tc.tile_pool(name="wz", bufs=2) as wzp, \
             tc.tile_pool(name="ht", bufs=2) as htp, \
             tc.tile_pool(name="yout", bufs=2) as yp, \
             tc.tile_pool(name="ps_g", bufs=2, space="PSUM") as psg, \
             tc.tile_pool(name="ps_r", bufs=1, space="PSUM") as psr, \
             tc.tile_pool(name="ps_f", bufs=2, space="PSUM") as psf, \
             tc.tile_pool(name="ps_y", bufs=3, space="PSUM") as psy:

            # constants
            iota_f = cp.tile([128, 1], dt.float32, tag="iota_f")
            iota_i = cp.tile([128, 1], dt.int32, tag="iota_i")
            nc.gpsimd.iota(iota_i[:], pattern=[[0, 1]], base=0, channel_multiplier=1)
            nc.vector.tensor_copy(iota_f[:], iota_i[:])
            ones128 = cp.tile([128, 1], dt.float32, tag="ones128")
            nc.vector.memset(ones128[:], 1.0)
            ones1 = cp.tile([1, 128], dt.float32, tag="ones1")
            nc.vector.memset(ones1[:], 1.0)
            # identity matrix built on device: row-iota == partition-iota
            rowi_i = cp.tile([128, 128], dt.int32, tag="rowi_i")
            nc.gpsimd.iota(rowi_i[:], pattern=[[1, 128]], base=0,
                           channel_multiplier=0)
            rowi_f = cp.tile([128, 128], dt.float32, tag="rowi_f")
            nc.vector.tensor_copy(rowi_f[:], rowi_i[:])
            id16 = cp.tile([128, 128], dt.float16, tag="id16")
            nc.vector.tensor_scalar(out=id16[:], in0=rowi_f[:],
                                    scalar1=iota_f[:, 0:1], scalar2=None,
                                    op0=ALU.is_equal)

            iota2 = cp.tile([128, 2], dt.float32, tag="iota2")
            nc.vector.tensor_copy(iota2[:, 0:1], iota_f[:])
            nc.vector.tensor_scalar_add(iota2[:, 1:2], iota_f[:], float(E * 128))

            for rep in range(reps):
                loads = []
                for s in range(SPC):
                    gwt = gp.tile([128, CK, 16], dt.float16, tag="gwt",
                                  name=f"gwt{s}")
                    epst = gp.tile([128, TCH, 8], dt.float16, tag="epst",
                                   name=f"epst{s}")
                    loads.append((gwt, epst))
                xt16s = [xtp.tile([128, CK, N], dt.float16, tag="xt16",
                                  name=f"xt16_{s}") for s in range(SPC)]
                xt8s = [xtp.tile([128, CK, N], dt.float8e4, tag="xt8",
                                 name=f"xt8_{s}") for s in range(SPC)]
                # s0 gating inputs first (tiny, then x pieces so gating can
                # start after the first quarter); s1 tiny inputs follow.
                nc.sync.dma_start(loads[0][0][:], gw_d[0])
                nc.sync.dma_start(loads[0][1][:], ep_d[0])
                src16_0 = xt16_d[0].rearrange("k p n -> p k n")
                for pc in range(4):
                    nsl = slice(256 * pc, 256 * (pc + 1))
                    nc.sync.dma_start(xt16s[0][:, :, nsl], src16_0[:, :, nsl])
                nc.sync.dma_start(loads[1][0][:], gw_d[1])
                nc.sync.dma_start(loads[1][1][:], ep_d[1])
                nc.sync.dma_start(xt8s[0][:],
                                  xt8_d[0].rearrange("k p n -> p k n"))

                states = []
                for s in range(SPC):
                    gwt, epst = loads[s]
                    xt16 = xt16s[s]
                    xt8 = xt8s[s]

                    # ---- gating logits: [128 tok, 16] per token chunk ----
                    # (one full-region accumulation group per PSUM tile)
                    comb = gp.tile([128, TCH, 16], dt.float32, tag="comb")
                    for t in range(TCH):
                        g_ps = psg.tile([128, 16], dt.float32, space="PSUM", tag="g")
                        for k in range(CK):
                            nc.tensor.matmul(
                                out=g_ps[:],
                                lhsT=xt16[:, k, 128 * t:128 * (t + 1)],
                                rhs=gwt[:, k, :],
                                start=(k == 0), stop=(k == CK - 1))
                        nc.vector.tensor_copy(comb[:, t, :], g_ps[:])

                    # ---- noise: comb[:, :, 8:16] = (softplus(raw)+0.01)*eps16 ----
                    # softplus via tanh model (single act table); raw16 = 16*raw
                    r16v = comb[:, :, 8:16]
                    th = gp.tile([128, TCH, 8], dt.float32, tag="th")
                    nc.scalar.activation(th[:], r16v, AF.Tanh,
                                         scale=SP_C2 / 32.0)
                    av = gp.tile([128, TCH, 8], dt.float32, tag="av")
                    nc.vector.tensor_scalar(out=av[:], in0=r16v,
                                            scalar1=SP_C3 / 1024.0,
                                            scalar2=1.0 / 32.0,
                                            op0=ALU.mult, op1=ALU.add)
                    vv = gp.tile([128, TCH, 8], dt.float32, tag="vv")
                    nc.vector.tensor_tensor(out=vv[:], in0=av[:], in1=r16v,
                                            op=ALU.mult)
                    wv = gp.tile([128, TCH, 8], dt.float32, tag="wv")
                    nc.vector.tensor_tensor(out=wv[:], in0=r16v, in1=th[:],
                                            op=ALU.mult)
                    spt = gp.tile([128, TCH, 8], dt.float32, tag="spt")
                    nc.vector.tensor_scalar(out=spt[:], in0=wv[:],
                                            scalar1=SP_C1 / 32.0,
                                            scalar2=SP_C0 + 0.01,
                                            op0=ALU.mult, op1=ALU.add)
                    nc.vector.tensor_tensor(out=spt[:], in0=spt[:], in1=vv[:],
                                            op=ALU.add)
                    nc.vector.tensor_tensor(out=comb[:, :, 8:16], in0=spt[:],
                                            in1=epst[:], op=ALU.mult)

                    # ---- token reduction via ones-matmul; ews = clean+noise ----
                    rb_ps = psr.tile([128, 24], dt.float32, space="PSUM", tag="rb")
                    for t in range(TCH):
                        nc.tensor.matmul(out=rb_ps[0:1, 0:16], lhsT=ones128[:],
                                         rhs=comb[:, t, :],
                                         start=(t == 0), stop=(t == TCH - 1),
                                         skip_group_check=True)
                    rrow = gp.tile([1, 16], dt.float32, tag="rrow")
                    nc.vector.tensor_copy(rrow[:], rb_ps[0:1, 0:16])
                    ews = gp.tile([1, 8], dt.float32, tag="ews")
                    nc.vector.tensor_tensor(out=ews[:], in0=rrow[0:1, 0:8],
                                            in1=rrow[0:1, 8:16], op=ALU.add)
                    nc.tensor.matmul(out=rb_ps[:, 16:24], lhsT=ones1[:], rhs=ews[:],
                                     start=True, stop=True, skip_group_check=True)
                    ewsb = gp.tile([128, 8], dt.float32, tag="ewsb")
                    nc.vector.tensor_copy(ewsb[:], rb_ps[:, 16:24])
                    mx = gp.tile([128, 8], dt.float32, tag="mx")
                    mi = gp.tile([128, 8], dt.uint32, tag="mi")
                    nc.vector.max_with_indices(mx[:], mi[:], ewsb[:])

                    # ---- gather packed weights for top-1 (G1 copy) / top-2
                    # (G2 copy, rows offset by E*128) in ONE indirect DMA ----
                    mif = gp.tile([128, 2], dt.float32, tag="mif")
                    nc.vector.tensor_copy(mif[:], mi[:, 0:2])
                    bf = gp.tile([128, 2], dt.float32, tag="bf")
                    nc.vector.tensor_scalar(out=bf[:], in0=mif[:],
                                            scalar1=128.0, scalar2=None,
                                            op0=ALU.mult)
                    nc.vector.tensor_tensor(out=bf[:], in0=bf[:], in1=iota2[:],
                                            op=ALU.add)
                    gi = gp.tile([128, 2], dt.uint32, tag="gi")
                    nc.vector.tensor_copy(gi[:], bf[:])
                    wts = []
                    for rk in range(TOPK):
                        wt = wzp.tile([128, 24 * 128], dt.float8e4, tag=f"wt{rk}")
                        nc.gpsimd.indirect_dma_start(
                            out=wt[:], out_offset=None, in_=wp_d[:],
                            in_offset=bass.IndirectOffsetOnAxis(
                                ap=gi[:, rk:rk + 1], axis=0))
                        wts.append(wt[:].rearrange("p (q i) -> p q i", i=128))
                    if s == 0:
                        # s1 bulk loads issue on the Pool queue BEHIND the s0
                        # gathers, so the gathers reach the DMA engines first
                        src16_1 = xt16_d[1].rearrange("k p n -> p k n")
                        for pc in range(2):
                            nsl = slice(512 * pc, 512 * (pc + 1))
                            nc.gpsimd.dma_start(xt16s[1][:, :, nsl],
                                                src16_1[:, :, nsl])
                        nc.gpsimd.dma_start(xt8s[1][:],
                                            xt8_d[1].rearrange("k p n -> p k n"))
                    if dbg:
                        nc.sync.dma_start(dcomb_d[s], comb[:])
                        nc.sync.dma_start(dews_d[s], ewsb[:])
                        nc.sync.dma_start(dmi_d[s], mi[:])
                        for rk in range(TOPK):
                            nc.sync.dma_start(dwt_d[s, rk],
                                              wts[rk].rearrange("p q i -> p (q i)"))
                    states.append((xt16, xt8, wts))

                for s in range(SPC):
                    xt16, xt8, wts = states[s]
                    # ---- fc1 (DoubleRow fp8) + gelu -> hT fp8 ----
                    hts = []
                    for rk in range(TOPK):
                        wt = wts[rk]
                        ht = htp.tile([128, 2, N], dt.float8e4, tag=f"ht{rk}")
                        for m in range(2):
                            for n in range(NT):
                                f_ps = psf.tile([128, 512], dt.float32,
                                                space="PSUM", tag="f")
                                for j in range(3):
                                    nc.tensor.matmul(
                                        out=f_ps[:],
                                        lhsT=wt[:, m * 6 + 2 * j:m * 6 + 2 * j + 2, :],
                                        rhs=xt8[:, 2 * j:2 * j + 2,
                                                512 * n:512 * (n + 1)],
                                        start=(j == 0), stop=(j == 2),
                                        perf_mode=PM.DoubleRow)
                                nc.scalar.activation(
                                    ht[:, m, 512 * n:512 * (n + 1)], f_ps[:],
                                    AF.Gelu)
                        hts.append(ht)
                        if dbg:
                            nc.sync.dma_start(dht_d[s, rk], ht[:])

                    # ---- fc2 + residual (identity matmul), out [C, N] fp16 ----
                    ys = yp.tile([128, CK, N], dt.float16, tag="ys")
                    for c in range(CK):
                        for n in range(NT):
                            y_ps = psy.tile([128, 512], dt.float32,
                                            space="PSUM", tag="y")
                            nc.tensor.matmul(
                                out=y_ps[:], lhsT=id16[:],
                                rhs=xt16[:, c, 512 * n:512 * (n + 1)],
                                start=True, stop=False)
                            for rk in range(TOPK):
                                nc.tensor.matmul(
                                    out=y_ps[:],
                                    lhsT=wts[rk][:, 12 + 2 * c:12 + 2 * c + 2, :],
                                    rhs=hts[rk][:, :, 512 * n:512 * (n + 1)],
                                    start=False, stop=(rk == TOPK - 1),
                                    perf_mode=PM.DoubleRow)
                            dst = ys[:, c, 512 * n:512 * (n + 1)]
                            if (c * NT + n) % 2 == 0:
                                nc.vector.tensor_copy(dst, y_ps[:])
                            else:
                                nc.scalar.activation(dst, y_ps[:], AF.Copy)
                        if c % 2 == 1:
                            nc.sync.dma_start(
                                y_d[s, c - 1:c + 1].rearrange("k p n -> p k n"),
                                ys[:, c - 1:c + 1, :])

    nc.compile()
    _cache[key] = nc
    return nc


def _prep_inputs(x, task_ids, eps, gate_w, fc1_w, fc1_b, fc2_w, fc2_b):
    x = np.ascontiguousarray(np.asarray(x, dtype=f32))
    task_ids = np.asarray(task_ids).astype(np.int64)
    eps = np.asarray(eps, dtype=f32)
    gate_w = np.asarray(gate_w, dtype=f32)
    fc1_w = np.asarray(fc1_w, dtype=f32)
    fc2_w = np.asarray(fc2_w, dtype=f32)
    fc1_b = np.asarray(fc1_b, dtype=f32)
    fc2_b = np.asarray(fc2_b, dtype=f32)
    assert not fc1_b.any() and not fc2_b.any(), "nonzero biases unsupported"

    # xT [B, CK, 128, N] in fp16 and fp8 (both quantized from f32 x)
    xT = np.ascontiguousarray(np.swapaxes(x, 1, 2)).reshape(B, CK, 128, N)
    xt16 = xT.astype(f16)
    xt8 = xT.astype(e4)

    # gating weights: [B, 128, CK*16] = 16*gate_w[task][c=128k+p, j]
    gw = (16.0 * gate_w[task_ids]).reshape(B, CK, 128, 2 * E)
    gw16 = np.ascontiguousarray(gw.transpose(0, 2, 1, 3)).reshape(B, 128, CK * 16).astype(f16)

    # eps: [B, 128, TCH*8] = 16*eps[n=128t+p, e]
    ep = (16.0 * eps).reshape(B, TCH, 128, E)
    eps16 = np.ascontiguousarray(ep.transpose(0, 2, 1, 3)).reshape(B, 128, TCH * 8).astype(f16)

    # packed weights [2E*128, 24*128] fp8:
    #  fc1 blocks q = m*6 + 2j + kk : w1[e, m*128+i, (2j+kk)*128+p]  (m=1,i>=64 -> 0)
    #  fc2 blocks q = 12 + c*2 + j  : G_rk*w2[e, c*128+i, j*128+p]   (j=1,p>=64 -> 0)
    w1p = np.zeros((E, 128, 2, CK, 128), dtype=f32)        # [e, p, m, k, i]
    w1t = np.swapaxes(fc1_w, 1, 2).reshape(E, CK, 128, H)  # [e, k, p, h]
    w1p[:, :, 0, :, :] = w1t[:, :, :, 0:128].transpose(0, 2, 1, 3)
    w1p[:, :, 1, :, 0:64] = w1t[:, :, :, 128:H].transpose(0, 2, 1, 3)
    # reorder to col layout q = m*6 + 2j + kk -> [e, p, m, j, kk, i] with k=2j+kk
    w1cols = w1p.reshape(E, 128, 2, 3, 2, 128)             # k -> (j, kk)
    fc1_flat = w1cols.reshape(E, 128, 12 * 128)

    w2p = np.zeros((E, 128, CK, 2, 128), dtype=f32)        # [e, p, c, j, i]
    w2t = np.swapaxes(fc2_w, 1, 2)                         # [e, h, c]
    w2t_pad = np.zeros((E, 256, C), dtype=f32)
    w2t_pad[:, 0:H, :] = w2t
    w2v = w2t_pad.reshape(E, 2, 128, CK, 128)              # [e, j, p, c, i]
    w2p[:] = w2v.transpose(0, 2, 3, 1, 4)
    fc2_flat = w2p.reshape(E, 128, 12 * 128)

    wpack = np.zeros((2, E, 128, PCK), dtype=f32)
    for rk, g in enumerate((G1, G2)):
        wpack[rk, :, :, 0:12 * 128] = fc1_flat
        wpack[rk, :, :, 12 * 128:] = g * fc2_flat
    wpack = wpack.reshape(2 * E * 128, PCK).astype(e4)

    in_maps = []
    for cc in range(NCORES):
        sl = slice(SPC * cc, SPC * (cc + 1))
        in_maps.append({
            "xt16": xt16[sl], "xt8": xt8[sl],
            "gw16": gw16[sl], "eps16": eps16[sl],
            "wpack": wpack,
        })
    return in_maps


def kernel(x, task_ids, eps, gate_w, fc1_w, fc1_b, fc2_w, fc2_b, _trace=False):
    nc = _build()
    in_maps = _prep_inputs(x, task_ids, eps, gate_w, fc1_w, fc1_b, fc2_w, fc2_b)
    res = run_bass_kernel_spmd(nc, in_maps, list(range(NCORES)), trace=_trace)
    outs = []
    for cc in range(NCORES):
        yT = res.results[cc]["yT"]                      # [SPC, CK, 128, N] f16
        y = yT.astype(f32).transpose(0, 3, 1, 2).reshape(SPC, N, C)
        outs.append(y)
    kernel.last_results = res
    return np.concatenate(outs, axis=0)


# revision 55
# speedup vs baseline: 1.1600x; 1.1600x over previous
"""MoE block (B=16,N=1024,C=768,E=8,H=192,D=4,K=2) on 8 NeuronCores.

Strategy: data-parallel over B (2 samples/core). Everything is laid out to
minimize DMA bytes/instructions (the cost-model bottleneck) and PE column
traffic:

  - xT fp16 (pre-transposed on host) serves gating (needs ~11 mantissa bits
    for exact top-2), the fc2-side residual add, and is the only dense fp16
    copy of x. A second fp8(e4m3) copy feeds fc1 in DoubleRow mode.
  - Gating matmuls use tiny output columns (out [128 tokens, 16]) so PE cost
    is ~16 cols/chunk instead of 512. Token reduction of clean/noise logits
    is a ones-vector matmul accumulated in PSUM.
  - Top-2 gate VALUES are constants (softmax of (d)/(d+1e-6) over 2 entries
    saturates: g1=sigmoid(1), g2=1-g1, exact to <1e-6 for any non-degenerate
    gap), so they are folded into two pre-scaled fc2 copies in the packed
    weight table; the top-1/top-2 gathers select the right copy. This removes
    all per-element gate multiplies.
  - fc1/fc2 run in fp8e4 DoubleRow (0.5 cyc/col, 256-deep contraction);
    gelu is applied PSUM->fp8 hT directly on the scalar engine.
  - Output is computed in transposed [C, N] layout so the residual is one
    more matmul (identity x xT16) accumulated into the fc2 PSUM; host
    transposes back. Output dtype fp16.
"""
import numpy as np
import ml_dtypes

import concourse.bass as bass
import concourse.bass_isa as bass_isa
import concourse.mybir as mybir
import concourse.tile as tile
from concourse import bacc
from concourse.bass_utils import run_bass_kernel_spmd

f16 = np.float16
f32 = np.float32
e4 = ml_dtypes.float8_e4m3
AF = mybir.ActivationFunctionType
ALU = mybir.AluOpType
PM = mybir.MatmulPerfMode
dt = mybir.dt

B, N, C = 16, 1024, 768
E, H, D, TOPK = 8, 192, 4, 2
NCORES = 8
SPC = B // NCORES          # samples per core = 2
CK = C // 128              # 6 channel chunks
TCH = N // 128             # 8 token chunks
NT = N // 512              # 2 n-chunks for the 512-wide MLP matmuls
PCK = 24 * 128             # packed weight row: 12 fc1 blocks + 12 fc2 blocks
G1 = float(1.0 / (1.0 + np.exp(-1.0)))
G2 = 1.0 - G1
# softplus(r) = r/2 + g(r/2), g(y)=ln(2cosh y) ~= C0 + C1*y*tanh(C2*y) + C3*y^2
# (fit on |y|<=1.8, max err 2.3e-4; raw logits here stay within |y|<=1.25).
# Keeps the scalar engine on the single gelu table (tanh lives there too).
SP_C0, SP_C1, SP_C2, SP_C3 = (0.6932338862378958, 0.5501889808219406,
                              0.7575131375050952, 0.08185888665593381)

_cache = {}


def _build(reps=1, dbg=False):
    key = ("nc", reps, dbg)
    if key in _cache:
        return _cache[key]
    nc = bacc.Bacc("TRN2", target_bir_lowering=False, debug=False,
                   num_devices=NCORES)

    xt16_d = nc.dram_tensor("xt16", [SPC, CK, 128, N], dt.float16, kind="ExternalInput").ap()
    xt8_d = nc.dram_tensor("xt8", [SPC, CK, 128, N], dt.float8e4, kind="ExternalInput").ap()
    gm_d = nc.dram_tensor("gmix", [SPC, 128, CK * 16 + TCH * 8], dt.float16, kind="ExternalInput").ap()
    wp_d = nc.dram_tensor("wpack", [2 * E * 128, PCK], dt.float8e4, kind="ExternalInput").ap()
    y_d = nc.dram_tensor("yT", [SPC, CK, 128, N], dt.float16, kind="ExternalOutput").ap()
    if dbg:
        dcomb_d = nc.dram_tensor("dcomb", [SPC, 128, TCH * 16], dt.float32, kind="ExternalOutput").ap()
        dews_d = nc.dram_tensor("dews", [SPC, 128, 8], dt.float32, kind="ExternalOutput").ap()
        dmi_d = nc.dram_tensor("dmi", [SPC, 128, 8], dt.uint32, kind="ExternalOutput").ap()
        dwt_d = nc.dram_tensor("dwt", [SPC, TOPK, 128, PCK], dt.float8e4, kind="ExternalOutput").ap()
        dht_d = nc.dram_tensor("dht", [SPC, TOPK, 128, 2 * N], dt.float8e4, kind="ExternalOutput").ap()

    with tile.TileContext(nc) as tc:
        with tc.tile_pool(name="const", bufs=1) as cp, \
             tc.tile_pool(name="xt", bufs=2) as xtp, \
             tc.tile_pool(name="gate", bufs=2) as gp, \
             tc.tile_pool(name="wz", bufs=2) as wzp, \
             tc.tile_pool(name="ht", bufs=2) as htp, \
             tc.tile_pool(name="yout", bufs=2) as yp, \
             tc.tile_pool(name="ps_g", bufs=2, space="PSUM") as psg, \
             tc.tile_pool(name="ps_f", bufs=3, space="PSUM") as psf, \
             tc.tile_pool(name="ps_y", bufs=3, space="PSUM") as psy:

            # constants
            iota_f = cp.tile([128, 1], dt.float32, tag="iota_f")
            iota_i = cp.tile([128, 1], dt.int32, tag="iota_i")
            nc.gpsimd.iota(iota_i[:], pattern=[[0, 1]], base=0, channel_multiplier=1)
            nc.vector.tensor_copy(iota_f[:], iota_i[:])
            ones1 = cp.tile([1, 128], dt.float32, tag="ones1")
            nc.vector.memset(ones1[:], 1.0)
            # identity matrix built on device: row-iota == partition-iota
            rowi_i = cp.tile([128, 128], dt.int32, tag="rowi_i")
            nc.gpsimd.iota(rowi_i[:], pattern=[[1, 128]], base=0,
                           channel_multiplier=0)
            rowi_f = cp.tile([128, 128], dt.float32, tag="rowi_f")
            nc.vector.tensor_copy(rowi_f[:], rowi_i[:])
            id16 = cp.tile([128, 128], dt.float16, tag="id16")
            nc.vector.tensor_scalar(out=id16[:], in0=rowi_f[:],
                                    scalar1=iota_f[:, 0:1], scalar2=None,
                                    op0=ALU.is_equal)

            iota2 = cp.tile([128, 2], dt.float32, tag="iota2")
            nc.vector.tensor_copy(iota2[:, 0:1], iota_f[:])
            nc.vector.tensor_scalar_add(iota2[:, 1:2], iota_f[:], float(E * 128))
            # dummy gelu so the (only) activation table loaded is the gelu
            # set, which also contains tanh — avoids a mid-kernel table swap
            dum = cp.tile([1, 1], dt.float32, tag="dum")
            nc.scalar.activation(dum[:], ones1[0:1, 0:1], AF.Gelu)

            for rep in range(reps):
                loads = []
                for s in range(SPC):
                    gmix = gp.tile([128, CK * 16 + TCH * 8], dt.float16,
                                   tag="gmix", name=f"gmix{s}")
                    gwt = gmix[:, 0:CK * 16].rearrange("p (k j) -> p k j", j=16)
                    epst = gmix[:, CK * 16:].rearrange("p (t e) -> p t e", e=8)
                    loads.append((gmix, gwt, epst))
                xt16s = [xtp.tile([128, CK, N], dt.float16, tag="xt16",
                                  name=f"xt16_{s}") for s in range(SPC)]
                xt8s = [xtp.tile([128, CK, N], dt.float8e4, tag="xt8",
                                 name=f"xt8_{s}") for s in range(SPC)]
                # s0 gating inputs first (tiny, then x pieces so gating can
                # start after the first quarter); s1 follows.
                nc.sync.dma_start(loads[0][0][:], gm_d[0])
                src16_0 = xt16_d[0].rearrange("k p n -> p k n")
                for pc in range(4):
                    nsl = slice(256 * pc, 256 * (pc + 1))
                    nc.sync.dma_start(xt16s[0][:, :, nsl], src16_0[:, :, nsl])
                nc.sync.dma_start(loads[1][0][:], gm_d[1])
                src16_1 = xt16_d[1].rearrange("k p n -> p k n")
                for pc in range(2):
                    nsl = slice(512 * pc, 512 * (pc + 1))
                    nc.sync.dma_start(xt16s[1][:, :, nsl], src16_1[:, :, nsl])
                nc.sync.dma_start(xt8s[0][:],
                                  xt8_d[0].rearrange("k p n -> p k n"))
                nc.sync.dma_start(xt8s[1][:],
                                  xt8_d[1].rearrange("k p n -> p k n"))

                states = []
                for s in range(SPC):
                    gmix, gwt, epst = loads[s]
                    xt16 = xt16s[s]
                    xt8 = xt8s[s]

                    # gating logits [128 tok, 16] per token chunk; 4 chunk
                    # regions per PSUM tile (starts zero only written bytes)
                    comb = gp.tile([128, TCH, 16], dt.float32, tag="comb",
                                   name=f"comb{s}")
                    for th_ in range(2):
                        g_ps = psg.tile([128, 4, 16], dt.float32, space="PSUM",
                                        tag="g", name=f"g{s}{th_}")
                        for tt_ in range(4):
                            t = 4 * th_ + tt_
                            for k in range(CK):
                                nc.tensor.matmul(
                                    out=g_ps[:, tt_, :],
                                    lhsT=xt16[:, k, 128 * t:128 * (t + 1)],
                                    rhs=gwt[:, k, :],
                                    start=(k == 0), stop=(k == CK - 1),
                                    skip_group_check=True)
                        nc.vector.tensor_copy(comb[:, 4 * th_:4 * (th_ + 1), :],
                                              g_ps[:])

                    # noise: comb[:, :, 8:16] = (softplus(raw)+0.01)*eps16,
                    # softplus via tanh model (single act table); raw16=16*raw
                    r16v = comb[:, :, 8:16]
                    th = gp.tile([128, TCH, 8], dt.float32, tag="th",
                                 name=f"th{s}")
                    nc.scalar.activation(th[:], r16v, AF.Tanh,
                                         scale=SP_C2 / 32.0)
                    av = gp.tile([128, TCH, 8], dt.float32, tag="av",
                                 name=f"av{s}")
                    nc.vector.tensor_scalar(out=av[:], in0=r16v,
                                            scalar1=SP_C3 / 1024.0,
                                            scalar2=1.0 / 32.0,
                                            op0=ALU.mult, op1=ALU.add)
                    vv = gp.tile([128, TCH, 8], dt.float32, tag="vv",
                                 name=f"vv{s}")
                    nc.vector.tensor_tensor(out=vv[:], in0=av[:], in1=r16v,
                                            op=ALU.mult)
                    wv = gp.tile([128, TCH, 8], dt.float32, tag="wv",
                                 name=f"wv{s}")
                    nc.vector.tensor_tensor(out=wv[:], in0=r16v, in1=th[:],
                                            op=ALU.mult)
                    spt = gp.tile([128, TCH, 8], dt.float32, tag="spt",
                                  name=f"spt{s}")
                    nc.vector.tensor_scalar(out=spt[:], in0=wv[:],
                                            scalar1=SP_C1 / 32.0,
                                            scalar2=SP_C0 + 0.01,
                                            op0=ALU.mult, op1=ALU.add)
                    nc.vector.tensor_tensor(out=spt[:], in0=spt[:], in1=vv[:],
                                            op=ALU.add)
                    nc.vector.tensor_tensor(out=comb[:, :, 8:16], in0=spt[:],
                                            in1=epst, op=ALU.mult)

                    # token reduction: all partitions via GPSIMD, then the
                    # chunk axis on DVE; no PE involvement in the gating tail
                    red = gp.tile([128, TCH * 16], dt.float32, tag="red",
                                  name=f"red{s}")
                    nc.gpsimd.partition_all_reduce(
                        red[:], comb[:].rearrange("p t j -> p (t j)"),
                        channels=128, reduce_op=bass_isa.ReduceOp.add)
                    ewsum = gp.tile([128, 16, 1], dt.float32, tag="ewsum",
                                    name=f"ewsum{s}")
                    nc.vector.tensor_reduce(
                        out=ewsum[:], in_=red[:].rearrange("p (t j) -> p j t", j=16),
                        axis=mybir.AxisListType.X, op=ALU.add)
                    ews8 = gp.tile([128, 8], dt.float32, tag="ews8",
                                   name=f"ews8{s}")
                    nc.vector.tensor_tensor(out=ews8[:], in0=ewsum[:, 0:8, 0],
                                            in1=ewsum[:, 8:16, 0], op=ALU.add)
                    mx = gp.tile([128, 8], dt.float32, tag="mx", name=f"mx{s}")
                    mi = gp.tile([128, 8], dt.uint32, tag="mi", name=f"mi{s}")
                    nc.vector.max_with_indices(mx[:], mi[:], ews8[:])

                    # ---- gather row ids: top-1 -> G1 copy rows, top-2 -> G2
                    # copy rows (offset E*128) ----
                    bf = gp.tile([128, 2], dt.float32, tag="bf")
                    nc.vector.tensor_scalar(out=bf[:], in0=mi[:, 0:2],
                                            scalar1=128.0, scalar2=None,
                                            op0=ALU.mult)
                    gi = gp.tile([128, 2], dt.uint32, tag="gi")
                    nc.vector.tensor_tensor(out=gi[:], in0=bf[:], in1=iota2[:],
                                            op=ALU.add)
                    wts = []
                    for rk in range(TOPK):
                        wt = wzp.tile([128, 24 * 128], dt.float8e4, tag=f"wt{rk}")
                        nc.gpsimd.indirect_dma_start(
                            out=wt[:], out_offset=None, in_=wp_d[:],
                            in_offset=bass.IndirectOffsetOnAxis(
                                ap=gi[:, rk:rk + 1], axis=0))
                        wts.append(wt[:].rearrange("p (q i) -> p q i", i=128))
                    if dbg:
                        nc.sync.dma_start(dcomb_d[s], comb[:])
                        nc.sync.dma_start(dews_d[s], mx[:])
                        nc.sync.dma_start(dmi_d[s], mi[:])
                        for rk in range(TOPK):
                            nc.sync.dma_start(dwt_d[s, rk],
                                              wts[rk].rearrange("p q i -> p (q i)"))
                    states.append((xt16, xt8, wts))

                for s in range(SPC):
                    xt16, xt8, wts = states[s]
                    # ---- fc1 (DoubleRow fp8) + gelu -> hT fp8 ----
                    hts = []
                    for rk in range(TOPK):
                        wt = wts[rk]
                        ht = htp.tile([128, 2, N], dt.float8e4, tag=f"ht{rk}")
                        for m in range(2):
                            f_ps = psf.tile([128, N], dt.float32,
                                            space="PSUM", tag="f")
                            for n in range(NT):
                                for j in range(3):
                                    nc.tensor.matmul(
                                        out=f_ps[:, 512 * n:512 * (n + 1)],
                                        lhsT=wt[:, m * 6 + 2 * j:m * 6 + 2 * j + 2, :],
                                        rhs=xt8[:, 2 * j:2 * j + 2,
                                                512 * n:512 * (n + 1)],
                                        start=(j == 0), stop=(j == 2),
                                        perf_mode=PM.DoubleRow,
                                        skip_group_check=True)
                            nc.scalar.activation(ht[:, m, :], f_ps[:], AF.Gelu)
                        hts.append(ht)
                        if dbg:
                            nc.sync.dma_start(dht_d[s, rk], ht[:])

                    # ---- fc2 + residual (identity matmul), out [C, N] fp16 ----
                    ys = yp.tile([128, CK, N], dt.float16, tag="ys")
                    for c in range(CK):
                        for n in range(NT):
                            y_ps = psy.tile([128, 512], dt.float32,
                                            space="PSUM", tag="y")
                            nc.tensor.matmul(
                                out=y_ps[:], lhsT=id16[:],
                                rhs=xt16[:, c, 512 * n:512 * (n + 1)],
                                start=True, stop=False)
                            for rk in range(TOPK):
                                nc.tensor.matmul(
                                    out=y_ps[:],
                                    lhsT=wts[rk][:, 12 + 2 * c:12 + 2 * c + 2, :],
                                    rhs=hts[rk][:, :, 512 * n:512 * (n + 1)],
                                    start=False, stop=(rk == TOPK - 1),
                                    perf_mode=PM.DoubleRow)
                            dst = ys[:, c, 512 * n:512 * (n + 1)]
                            if (c * NT + n) % 2 == 0:
                                nc.vector.tensor_copy(dst, y_ps[:])
                            else:
                                nc.scalar.activation(dst, y_ps[:], AF.Copy)
                        if c % 2 == 1:
                            nc.sync.dma_start(
                                y_d[s, c - 1:c + 1].rearrange("k p n -> p k n"),
                                ys[:, c - 1:c + 1, :])

    nc.compile()
    _cache[key] = nc
    return nc


def _prep_inputs(x, task_ids, eps, gate_w, fc1_w, fc1_b, fc2_w, fc2_b):
    x = np.ascontiguousarray(np.asarray(x, dtype=f32))
    task_ids = np.asarray(task_ids).astype(np.int64)
    eps = np.asarray(eps, dtype=f32)
    gate_w = np.asarray(gate_w, dtype=f32)
    fc1_w = np.asarray(fc1_w, dtype=f32)
    fc2_w = np.asarray(fc2_w, dtype=f32)
    fc1_b = np.asarray(fc1_b, dtype=f32)
    fc2_b = np.asarray(fc2_b, dtype=f32)
    assert not fc1_b.any() and not fc2_b.any(), "nonzero biases unsupported"

    # xT [B, CK, 128, N] in fp16 and fp8 (both quantized from f32 x)
    xT = np.ascontiguousarray(np.swapaxes(x, 1, 2)).reshape(B, CK, 128, N)
    xt16 = xT.astype(f16)
    xt8 = xT.astype(e4)

    # gating weights + eps merged: [B, 128, CK*16 + TCH*8]
    gw = (16.0 * gate_w[task_ids]).reshape(B, CK, 128, 2 * E)
    gw16 = gw.transpose(0, 2, 1, 3).reshape(B, 128, CK * 16)
    ep = (16.0 * eps).reshape(B, TCH, 128, E)
    eps16 = ep.transpose(0, 2, 1, 3).reshape(B, 128, TCH * 8)
    gmix = np.ascontiguousarray(
        np.concatenate([gw16, eps16], axis=2)).astype(f16)

    # packed weights [2E*128, 24*128] fp8:
    #  fc1 blocks q = m*6 + 2j + kk : w1[e, m*128+i, (2j+kk)*128+p]  (m=1,i>=64 -> 0)
    #  fc2 blocks q = 12 + c*2 + j  : G_rk*w2[e, c*128+i, j*128+p]   (j=1,p>=64 -> 0)
    w1p = np.zeros((E, 128, 2, CK, 128), dtype=f32)        # [e, p, m, k, i]
    w1t = np.swapaxes(fc1_w, 1, 2).reshape(E, CK, 128, H)  # [e, k, p, h]
    w1p[:, :, 0, :, :] = w1t[:, :, :, 0:128].transpose(0, 2, 1, 3)
    w1p[:, :, 1, :, 0:64] = w1t[:, :, :, 128:H].transpose(0, 2, 1, 3)
    # reorder to col layout q = m*6 + 2j + kk -> [e, p, m, j, kk, i] with k=2j+kk
    w1cols = w1p.reshape(E, 128, 2, 3, 2, 128)             # k -> (j, kk)
    fc1_flat = w1cols.reshape(E, 128, 12 * 128)

    w2p = np.zeros((E, 128, CK, 2, 128), dtype=f32)        # [e, p, c, j, i]
    w2t = np.swapaxes(fc2_w, 1, 2)                         # [e, h, c]
    w2t_pad = np.zeros((E, 256, C), dtype=f32)
    w2t_pad[:, 0:H, :] = w2t
    w2v = w2t_pad.reshape(E, 2, 128, CK, 128)              # [e, j, p, c, i]
    w2p[:] = w2v.transpose(0, 2, 3, 1, 4)
    fc2_flat = w2p.reshape(E, 128, 12 * 128)

    wpack = np.zeros((2, E, 128, PCK), dtype=f32)
    for rk, g in enumerate((G1, G2)):
        wpack[rk, :, :, 0:12 * 128] = fc1_flat
        wpack[rk, :, :, 12 * 128:] = g * fc2_flat
    wpack = wpack.reshape(2 * E * 128, PCK).astype(e4)

    in_maps = []
    for cc in range(NCORES):
        sl = slice(SPC * cc, SPC * (cc + 1))
        in_maps.append({
            "xt16": xt16[sl], "xt8": xt8[sl],
            "gmix": gmix[sl], "wpack": wpack,
        })
    return in_maps


def kernel(x, task_ids, eps, gate_w, fc1_w, fc1_b, fc2_w, fc2_b, _trace=False):
    nc = _build()
    in_maps = _prep_inputs(x, task_ids, eps, gate_w, fc1_w, fc1_b, fc2_w, fc2_b)
    res = run_bass_kernel_spmd(nc, in_maps, list(range(NCORES)), trace=_trace)
    outs = []
    for cc in range(NCORES):
        yT = res.results[cc]["yT"]                      # [SPC, CK, 128, N] f16
        y = yT.astype(f32).transpose(0, 3, 1, 2).reshape(SPC, N, C)
        outs.append(y)
    kernel.last_results = res
    return np.concatenate(outs, axis=0)
